# revision 4
# baseline (speedup 1.0000x reference)
"""Block self-attention (Gaussian kernel weights) Trainium2 Bass kernel, v4.

For each independent block of B=1024 rows of `features` [262144, 128]:
    w = exp(-(sq_i + sq_j - 2 x@x^T)/25.6);  out = (w @ x)/B
Blocks are data-parallel across 8 NeuronCores (32 blocks per core).

Key algebra: w = D_e A D_e with A = exp(2G/25.6) symmetric, e = exp(-sq/25.6).
  out_j = (e_j/B) * sum_i A_ij * (e_i x_i)
The diagonal i=j term equals x_j/B exactly (exponents cancel in fp32), so no
diag masking and no separate x/B add is needed.

v4 vs v3: dma_start_transpose costs a fixed ~1.25us of issue time on the sync
queue regardless of width, so v3's 8 transposes/block (7 mirror + tout) were
the bottleneck (326us serialized on Sync).  v4 stores A = exp(2G/25.6) in a
row-layout tile A6 [128, 6144]: rows 0-3 full width 1024 (upper-trapezoid
segments written by ACT, rest garbage), rows 4-7 at half width covering
j in [512,1024).  ONE transpose A6 -> AM [128,48,128] yields every mirror
chunk with mm2-friendly uniform strides (j-halves align with the row 0-3 /
4-7 split).  x^T also comes from one DMA transpose instead of 8 PE
transposes (frees ~100us of PE time + the trt PSUM bank -> 2 mm2 acc banks).
Sync transposes per block: 3 (xT, AM, outT).
"""

import math
import os

os.environ.setdefault("NEURON_RT_RESET_CORES", "1")

import numpy as np

import concourse.bass as bass
import concourse.tile as tile
from concourse import bacc, mybir
from concourse.bass_utils import run_bass_kernel_spmd

N_TOTAL = 262144
D = 128
B = 1024
NCORES = 8
ROWS_PER_CORE = N_TOTAL // NCORES   # 32768
NB_FULL = ROWS_PER_CORE // B        # 32 blocks per core
C = B // 128                        # 8 row-chunks per block

F32 = mybir.dt.float32
BF16 = mybir.dt.bfloat16
FP16 = mybir.dt.float16

SIGMA2X2 = 2.0 * (D / 10.0)         # 25.6
G_SCALE = 2.0 / SIGMA2X2            # 0.078125
NEG_INV = -1.0 / SIGMA2X2           # -0.0390625
# outT is cast fp32->fp16 with a 1/OSC scale to keep away from fp16 max;
# the tail multiplies by e_j*OSC/B.
OSC = 16.0

EXP = mybir.ActivationFunctionType.Exp
MULT = mybir.AluOpType.mult

# trapezoid packing in PSUM: row c covers cols [128c, 1024) => width 1024-128c
ROW_W = [B - 128 * c for c in range(C)]
CUM = [0]
for w in ROW_W:
    CUM.append(CUM[-1] + w)
PACK = CUM[C]                        # 4608
TILE_W = 1536
NT = PACK // TILE_W                  # 3 G-psum tiles per block

# A6 row-layout: rows 0-3 full [0,1024), rows 4-7 half [512,1024)
A6_W = 4 * 1024 + 4 * 512            # 6144
NK = A6_W // 128                     # 48 column-chunks -> AM [128, 48, 128]


def a6_off(c, j):
    """Offset of A[row c, col j] in the A6 [128, 6144] tile."""
    if c < 4:
        return 1024 * c + j
    assert j >= 512
    return 4096 + 512 * (c - 4) + (j - 512)


def mm1_pieces():
    """(tile_idx, off_in_tile, row_c, col_start, n) split at 512 banks."""
    ps = []
    for c in range(C):
        s = CUM[c]
        while s < CUM[c + 1]:
            e = min(CUM[c + 1], (s // 512 + 1) * 512)
            ps.append((s // TILE_W, s % TILE_W, c, 128 * c + (s - CUM[c]), e - s))
            s = e
    return ps


MM1_PIECES = mm1_pieces()            # 15 MMs
MM1_BY_TILE = [[p for p in MM1_PIECES if p[0] == t] for t in range(NT)]


def act_pieces():
    """(tile_idx, off_in_tile, row_c, col_start, n) split only at tile bounds."""
    ps = []
    for c in range(C):
        s = CUM[c]
        while s < CUM[c + 1]:
            e = min(CUM[c + 1], (s // TILE_W + 1) * TILE_W)
            ps.append((s // TILE_W, s % TILE_W, c, 128 * c + (s - CUM[c]), e - s))
            s = e
    return ps


ACT_PIECES = act_pieces()            # 10 ACT instrs per block
ACT_BY_TILE = [[p for p in ACT_PIECES if p[0] == t] for t in range(NT)]


def mm2_half_pieces(h):
    """MM pieces for j in [512h, 512h+512): list of (c, kind, js, je) in
    emission order, with per-piece (start, stop) accumulation flags."""
    lo, hi = 512 * h, 512 * h + 512
    out = []
    for c in range(C):
        # mirror: j in [0, 128c); direct: j in [128c, 1024)
        mjs, mje = max(0, lo), min(128 * c, hi)
        if mje > mjs:
            out.append((c, "mir", mjs, mje))
        djs, dje = max(128 * c, lo), min(B, hi)
        if dje > djs:
            out.append((c, "dir", djs, dje))
    flags = [(i == 0, i == len(out) - 1) for i in range(len(out))]
    return list(zip(out, flags))


MM2_HALF = [mm2_half_pieces(0), mm2_half_pieces(1)]


def build(nb: int = NB_FULL) -> bacc.Bacc:
    rows = nb * B
    nc = bacc.Bacc("TRN2", target_bir_lowering=False, debug=False)

    fin = nc.dram_tensor("features", [rows, D], F32, kind="ExternalInput").ap()
    fout = nc.dram_tensor("out", [rows, D], F32, kind="ExternalOutput").ap()

    # row index = b*1024 + c*128 + p
    fin_v = fin.rearrange("(b c p) d -> b p c d", p=128, c=C)
    fout_v = fout.rearrange("(b c p) d -> b p c d", p=128, c=C)

    with tile.TileContext(nc) as tc:
        with (
            tc.tile_pool(name="xr", bufs=3) as xrpool,
            tc.tile_pool(name="xt", bufs=2) as xtpool,
            tc.tile_pool(name="y", bufs=5) as ypool,
            tc.tile_pool(name="sq", bufs=4) as sqpool,
            tc.tile_pool(name="a6", bufs=3) as a6pool,    # A rows bf16
            tc.tile_pool(name="am", bufs=3) as ampool,    # mirror [128,48,128]
            tc.tile_pool(name="ot", bufs=2) as otpool,    # outT_sb fp16
            tc.tile_pool(name="tr", bufs=2) as trpool,    # trd fp16
            tc.tile_pool(name="of", bufs=2) as ofpool,    # out_final fp32
            tc.tile_pool(name="gp", bufs=2, space="PSUM") as gpool,
            tc.tile_pool(name="acc", bufs=2, space="PSUM") as accpool,
        ):
            state: dict[int, dict] = {}

            def load(b):
                xr = xrpool.tile([128, C, D], BF16)
                nc.gpsimd.dma_start(out=xr[:], in_=fin_v[b])  # SWDGE cast DMA
                state[b] = dict(xr=xr)

            def prep(b):
                st = state[b]
                xr = st["xr"]
                xsq = sqpool.tile([128, C * D], BF16, tag="xsq")
                nc.gpsimd.tensor_mul(
                    xsq[:], xr[:].rearrange("p c d -> p (c d)"),
                    xr[:].rearrange("p c d -> p (c d)"),
                )
                sqcol = sqpool.tile([128, C], F32, tag="sqc")
                nc.vector.tensor_reduce(
                    sqcol[:], xsq[:].rearrange("p (c d) -> p c d", d=D),
                    axis=mybir.AxisListType.X, op=mybir.AluOpType.add,
                )
                bias_col = sqpool.tile([128, C], F32, tag="bia")
                nc.vector.tensor_scalar_mul(bias_col[:], sqcol[:], NEG_INV)
                st["bias_col"] = bias_col

            def escalc(b):
                # escB[p,c,d] = exp(-sq[p,c]/25.6): one ACT instr on a
                # stride-0-broadcast input, fp16 out
                st = state[b]
                escB = ypool.tile([128, C, D], FP16, tag="escB")
                nc.scalar.activation(
                    escB[:],
                    st.pop("bias_col")[:].unsqueeze(2).broadcast_to([128, C, D]),
                    EXP,
                )
                st["escB"] = escB

            def ymul(b):
                st = state[b]
                y = ypool.tile([128, C, D], BF16, tag="y")
                nc.vector.tensor_mul(y[:], st["xr"][:], st["escB"][:])
                st["y"] = y

            def xtrans(b):
                # x^T per 128-chunk via one DMA xbar transpose
                st = state[b]
                xT = xtpool.tile([128, C, 128], BF16)
                nc.sync.dma_start_transpose(
                    out=xT[:], in_=st["xr"][:].rearrange("p c d -> p (c d)")
                )
                st["xT"] = xT

            def m1_tile(b, t):
                st = state[b]
                if t == 0:
                    st["g"] = {}
                    st["a6"] = a6pool.tile([128, A6_W], BF16, name="a6", tag="a6")
                g = gpool.tile([128, TILE_W], F32, tag="g")
                st["g"][t] = g
                xT = st["xT"][:].rearrange("p c d -> p (c d)")
                for (_, off, c, col, n) in MM1_BY_TILE[t]:
                    nc.tensor.matmul(
                        g[:, off:off + n],
                        lhsT=st["xT"][:, c, :],
                        rhs=xT[:, col:col + n],
                        start=True, stop=True,
                    )

            def act_tile(b, t):
                # exp(G) pieces of psum tile t -> A6 row segments
                st = state[b]
                g = st["g"].pop(t)
                for (_, off, c, col, n) in ACT_BY_TILE[t]:
                    lo = a6_off(c, col)
                    nc.scalar.activation(
                        st["a6"][:, lo:lo + n], g[:, off:off + n], EXP,
                        scale=G_SCALE,
                    )

            def mir(b):
                # every mirror chunk in ONE xbar transpose:
                # AM[p, k, d] = A6[d, 128k + p]
                st = state[b]
                am = ampool.tile([128, NK, 128], BF16, name="am", tag="am")
                nc.sync.dma_start_transpose(out=am[:], in_=st["a6"][:])
                st["am"] = am

            def mm2_half(b, h):
                st = state[b]
                if h == 0:
                    st["ot"] = otpool.tile([128, B], FP16, name="ot", tag="ot")
                o = accpool.tile([128, 512], F32, tag="o")
                st["o%d" % h] = o
                amL = st["am"][:, 0:32, :].rearrange("p (s t) d -> p s t d", t=C)
                amH = st["am"][:, 32:NK, :].rearrange("p (s t) d -> p s t d", t=4)
                for (c, kind, js, je), (start, stop) in MM2_HALF[h]:
                    if kind == "mir":
                        # [p, s, d] AP with non-mergeable strides; matmul
                        # accepts multi-dim free APs
                        if h == 0:
                            rhs = amL[:, js // 128:je // 128, c, :]
                        else:
                            rhs = amH[:, (js - 512) // 128:(je - 512) // 128,
                                      c - 4, :]
                    else:
                        lo = a6_off(c, js)
                        rhs = st["a6"][:, lo:lo + (je - js)]
                    nc.tensor.matmul(
                        o[:, js - 512 * h:je - 512 * h],
                        lhsT=st["y"][:, c, :],
                        rhs=rhs,
                        start=start, stop=stop,
                    )

            def cast_half(b, h):
                st = state[b]
                nc.vector.tensor_scalar_mul(
                    st["ot"][:, h * 512:(h + 1) * 512],
                    st.pop("o%d" % h)[:], 1.0 / OSC
                )

            def tout(b):
                st = state[b]
                trd = trpool.tile([128, C, 128], FP16)
                nc.sync.dma_start_transpose(out=trd[:], in_=st.pop("ot")[:])
                st["trd"] = trd

            def tail(b):
                # out = (trd * OSC/B) * e_j  (one fused STT)
                st = state[b]
                of = ofpool.tile([128, C, D], F32)
                nc.vector.scalar_tensor_tensor(
                    out=of[:], in0=st["trd"][:], scalar=float(OSC / B),
                    in1=st["escB"][:], op0=MULT, op1=MULT,
                )
                st["of"] = of

            def store(b):
                st = state.pop(b)
                nc.sync.dma_start(out=fout_v[b], in_=st["of"][:])

            # software pipeline: iteration k handles load(k), prep/xtrans(k-1),
            # m1/act/mir(k-2), mm2/epilogue(k-4)
            for k in range(nb + 4):
                bl, bp, bm, be = k, k - 1, k - 2, k - 4
                if bl < nb:
                    load(bl)
                if 0 <= bp < nb:
                    prep(bp)
                if 0 <= bm < nb:
                    m1_tile(bm, 0)
                    act_tile(bm, 0)
                if 0 <= be < nb:
                    mm2_half(be, 0)
                if 0 <= bm < nb:
                    m1_tile(bm, 1)
                    act_tile(bm, 1)
                if 0 <= be < nb:
                    cast_half(be, 0)
                if 0 <= bm < nb:
                    m1_tile(bm, 2)
                    act_tile(bm, 2)
                if 0 <= be < nb:
                    mm2_half(be, 1)
                    cast_half(be, 1)
                    tout(be)
                if 0 <= bm < nb:
                    mir(bm)
                if 0 <= bp < nb:
                    xtrans(bp)
                    escalc(bp)
                    ymul(bp)
                if 0 <= be < nb:
                    tail(be)
                    store(be)

    nc.compile()
    return nc


_CACHE: dict[int, bacc.Bacc] = {}


def _get_nc(nb: int = NB_FULL) -> bacc.Bacc:
    if nb not in _CACHE:
        _CACHE[nb] = build(nb)
    return _CACHE[nb]


def run(features: np.ndarray, nc: bacc.Bacc | None = None, **spmd_kwargs):
    """Shard rows across 8 cores, run, gather. Returns (out, BassKernelResults)."""
    features = np.ascontiguousarray(features, dtype=np.float32)
    assert features.shape == (N_TOTAL, D)
    if nc is None:
        nc = _get_nc()
    core_ids = list(range(NCORES))
    shards = np.split(features, NCORES, axis=0)
    in_maps = [{"features": s} for s in shards]
    res = run_bass_kernel_spmd(nc, in_maps, core_ids, **spmd_kwargs)
    out = np.concatenate([res.results[i]["out"] for i in range(NCORES)], axis=0)
    return out, res


def kernel(features: np.ndarray) -> np.ndarray:
    out, _ = run(features)
    return out


# revision 5
# speedup vs baseline: 1.3088x; 1.3088x over previous
"""Block self-attention (Gaussian kernel weights) Trainium2 Bass kernel, v5.

For each independent block of B=1024 rows of `features` [262144, 128]:
    w = exp(-(sq_i + sq_j - 2 x@x^T)/25.6);  out = (w @ x)/B
Blocks are data-parallel across 8 NeuronCores (32 blocks per core).

Key algebra: w = D_e A D_e with A = exp(2G/25.6) symmetric, e = exp(-sq/25.6).
  out_j = (e_j/B) * sum_i A_ij * (e_i x_i)
The diagonal i=j term equals x_j/B exactly (exponents cancel in fp32), so no
diag masking and no separate x/B add is needed.

Measured HW facts driving v5 (from v3/v4 traces):
 - dma_start_transpose occupies the issuing queue for ~max(1250ns, 1.17ns/col)
   -> batch transposes, and trim garbage columns.
 - PE matmul has ~zero per-instruction overhead (0.71 ns/col at any width)
   -> fragmenting mm2 into per-chunk pieces is free on the PE stream.
 - DMA engines process ~256B/descriptor at ~19ns -> total descriptor count
   (loads+stores+transpose cols) is a real ceiling; p-major row labeling
   makes HBM loads/stores 4KB-contiguous per partition.

Layout: block rows are relabeled row = b*1024 + 8p + c (p-major) so HBM
DMA is contiguous per partition.  A = exp(2G/25.6) upper trapezoid lives in
a compact row tile A6 [128, 5120] with row pitches 1024,1024,768,768,512,
512,256,256 (row c stored from col j0 = 128*(2*(c//2))/2... see J0 below).
ONE transpose of A6[:, 128:4864] (4736 cols) yields every strict-lower
mirror chunk in AM [128, 37, 128].  x^T comes from one DMA transpose.
Sync-queue transposes per block: 3 (xT 1.25us + AM 5.5us + outT 1.25us).
"""

import math
import os

os.environ.setdefault("NEURON_RT_RESET_CORES", "1")

import numpy as np

import concourse.bass as bass
import concourse.tile as tile
from concourse import bacc, mybir
from concourse.bass_utils import run_bass_kernel_spmd

N_TOTAL = 262144
D = 128
B = 1024
NCORES = 8
ROWS_PER_CORE = N_TOTAL // NCORES   # 32768
NB_FULL = ROWS_PER_CORE // B        # 32 blocks per core
C = B // 128                        # 8 row-chunks per block

F32 = mybir.dt.float32
BF16 = mybir.dt.bfloat16
FP16 = mybir.dt.float16

SIGMA2X2 = 2.0 * (D / 10.0)         # 25.6
G_SCALE = 2.0 / SIGMA2X2            # 0.078125
NEG_INV = -1.0 / SIGMA2X2           # -0.0390625
# outT is cast fp32->fp16 with a 1/OSC scale to keep away from fp16 max;
# the tail multiplies by e_j*OSC/B.
OSC = 16.0

EXP = mybir.ActivationFunctionType.Exp
MULT = mybir.AluOpType.mult

# trapezoid packing in PSUM: row c covers cols [128c, 1024) => width 1024-128c
ROW_W = [B - 128 * c for c in range(C)]
CUM = [0]
for w in ROW_W:
    CUM.append(CUM[-1] + w)
PACK = CUM[C]                        # 4608
TILE_W = 1536
NT = PACK // TILE_W                  # 3 G-psum tiles per block

# Compact A6 row-layout: row c stored for j in [J0[c], 1024)
J0 = [0, 0, 256, 256, 512, 512, 768, 768]
PITCH = [1024 - j for j in J0]       # 1024,1024,768,768,512,512,256,256
BASE = [0]
for pw in PITCH:
    BASE.append(BASE[-1] + pw)
A6_W = BASE[C]                       # 5120
# Mirror transpose source: skip row0 diag chunk and all of row 7
MIR_LO = 128
MIR_HI = BASE[6] + PITCH[6]          # 4864 (end of row 6)
NK = (MIR_HI - MIR_LO) // 128        # 37 chunks in AM


def a6_off(c, j):
    """Offset of A[row c, col j] in the compact A6 tile."""
    assert j >= J0[c]
    return BASE[c] + (j - J0[c])


def am_idx(t, s):
    """AM chunk index holding mirror chunk A[i in t, j in s] (t > s)."""
    col = BASE[s] + 128 * t - J0[s]
    assert MIR_LO <= col < MIR_HI
    return col // 128 - 1


def mm1_pieces():
    """(tile_idx, off_in_tile, row_c, col_start, n) split at 512 banks."""
    ps = []
    for c in range(C):
        s = CUM[c]
        while s < CUM[c + 1]:
            e = min(CUM[c + 1], (s // 512 + 1) * 512)
            ps.append((s // TILE_W, s % TILE_W, c, 128 * c + (s - CUM[c]), e - s))
            s = e
    return ps


MM1_PIECES = mm1_pieces()            # 15 MMs
MM1_BY_TILE = [[p for p in MM1_PIECES if p[0] == t] for t in range(NT)]


def act_pieces():
    """(tile_idx, off_in_tile, row_c, col_start, n) split only at tile bounds."""
    ps = []
    for c in range(C):
        s = CUM[c]
        while s < CUM[c + 1]:
            e = min(CUM[c + 1], (s // TILE_W + 1) * TILE_W)
            ps.append((s // TILE_W, s % TILE_W, c, 128 * c + (s - CUM[c]), e - s))
            s = e
    return ps


ACT_PIECES = act_pieces()            # 10 ACT instrs per block
ACT_BY_TILE = [[p for p in ACT_PIECES if p[0] == t] for t in range(NT)]


def mm2_half_pieces(h):
    """MM pieces for j in [512h, 512h+512): list of (c, kind, js, je) in
    emission order, with per-piece (start, stop) accumulation flags.
    Mirror pieces are emitted per 128-chunk (PE instr overhead ~ 0)."""
    lo, hi = 512 * h, 512 * h + 512
    out = []
    for c in range(C):
        # mirror: j in [0, 128c); direct: j in [128c, 1024)
        mjs, mje = max(0, lo), min(128 * c, hi)
        for s in range(mjs // 128, max(mjs, mje) // 128):
            out.append((c, "mir", 128 * s, 128 * (s + 1)))
        djs, dje = max(128 * c, lo), min(B, hi)
        if dje > djs:
            out.append((c, "dir", djs, dje))
    flags = [(i == 0, i == len(out) - 1) for i in range(len(out))]
    return list(zip(out, flags))


MM2_HALF = [mm2_half_pieces(0), mm2_half_pieces(1)]


def build(nb: int = NB_FULL) -> bacc.Bacc:
    rows = nb * B
    nc = bacc.Bacc("TRN2", target_bir_lowering=False, debug=False)

    fin = nc.dram_tensor("features", [rows, D], F32, kind="ExternalInput").ap()
    fout = nc.dram_tensor("out", [rows, D], F32, kind="ExternalOutput").ap()

    # p-major row labeling: row index = b*1024 + p*8 + c -> per-partition
    # HBM spans are 8*128*4B = 4KB contiguous
    fin_v = fin.rearrange("(b p c) d -> b p (c d)", p=128, c=C)
    fout_v = fout.rearrange("(b p c) d -> b p c d", p=128, c=C)

    with tile.TileContext(nc) as tc:
        with (
            tc.tile_pool(name="xr", bufs=3) as xrpool,
            tc.tile_pool(name="xt", bufs=2) as xtpool,
            tc.tile_pool(name="y", bufs=5) as ypool,
            tc.tile_pool(name="sq", bufs=4) as sqpool,
            tc.tile_pool(name="a6", bufs=3) as a6pool,    # A rows bf16
            tc.tile_pool(name="am", bufs=3) as ampool,    # mirror [128,37,128]
            tc.tile_pool(name="ot", bufs=2) as otpool,    # outT_sb fp16
            tc.tile_pool(name="tr", bufs=2) as trpool,    # trd fp16
            tc.tile_pool(name="of", bufs=2) as ofpool,    # out_final fp32
            tc.tile_pool(name="gp", bufs=2, space="PSUM") as gpool,
            tc.tile_pool(name="acc", bufs=2, space="PSUM") as accpool,
        ):
            state: dict[int, dict] = {}

            def load(b):
                xr = xrpool.tile([128, C, D], BF16)
                nc.gpsimd.dma_start(
                    out=xr[:].rearrange("p c d -> p (c d)"), in_=fin_v[b]
                )  # SWDGE cast DMA, 4KB/partition contiguous
                state[b] = dict(xr=xr)

            def prep(b):
                st = state[b]
                xr = st["xr"]
                xsq = sqpool.tile([128, C * D], BF16, tag="xsq")
                nc.gpsimd.tensor_mul(
                    xsq[:], xr[:].rearrange("p c d -> p (c d)"),
                    xr[:].rearrange("p c d -> p (c d)"),
                )
                sqcol = sqpool.tile([128, C], F32, tag="sqc")
                nc.vector.tensor_reduce(
                    sqcol[:], xsq[:].rearrange("p (c d) -> p c d", d=D),
                    axis=mybir.AxisListType.X, op=mybir.AluOpType.add,
                )
                bias_col = sqpool.tile([128, C], F32, tag="bia")
                nc.vector.tensor_scalar_mul(bias_col[:], sqcol[:], NEG_INV)
                st["bias_col"] = bias_col

            def escalc(b):
                # escB[p,c,d] = exp(-sq[p,c]/25.6): one ACT instr on a
                # stride-0-broadcast input, fp16 out
                st = state[b]
                escB = ypool.tile([128, C, D], FP16, tag="escB")
                nc.scalar.activation(
                    escB[:],
                    st.pop("bias_col")[:].unsqueeze(2).broadcast_to([128, C, D]),
                    EXP,
                )
                st["escB"] = escB

            def ymul(b):
                st = state[b]
                y = ypool.tile([128, C, D], BF16, tag="y")
                nc.vector.tensor_mul(y[:], st["xr"][:], st["escB"][:])
                st["y"] = y

            def xtrans(b):
                # x^T per 128-chunk via one DMA xbar transpose
                st = state[b]
                xT = xtpool.tile([128, C, 128], BF16)
                nc.sync.dma_start_transpose(
                    out=xT[:], in_=st["xr"][:].rearrange("p c d -> p (c d)")
                )
                st["xT"] = xT

            def m1_tile(b, t):
                st = state[b]
                if t == 0:
                    st["g"] = {}
                    st["a6"] = a6pool.tile([128, A6_W], BF16, name="a6", tag="a6")
                g = gpool.tile([128, TILE_W], F32, tag="g")
                st["g"][t] = g
                xT = st["xT"][:].rearrange("p c d -> p (c d)")
                for (_, off, c, col, n) in MM1_BY_TILE[t]:
                    nc.tensor.matmul(
                        g[:, off:off + n],
                        lhsT=st["xT"][:, c, :],
                        rhs=xT[:, col:col + n],
                        start=True, stop=True,
                    )

            def act_tile(b, t):
                # exp(G) pieces of psum tile t -> compact A6 row segments
                st = state[b]
                g = st["g"].pop(t)
                for (_, off, c, col, n) in ACT_BY_TILE[t]:
                    lo = a6_off(c, col)
                    nc.scalar.activation(
                        st["a6"][:, lo:lo + n], g[:, off:off + n], EXP,
                        scale=G_SCALE,
                    )

            def mir(b):
                # every strict-lower mirror chunk in ONE xbar transpose:
                # AM[p, k, d] = A6[d, 128 + 128k + p]
                st = state[b]
                am = ampool.tile([128, NK, 128], BF16, name="am", tag="am")
                nc.sync.dma_start_transpose(
                    out=am[:], in_=st["a6"][:, MIR_LO:MIR_HI]
                )
                st["am"] = am

            def mm2_half(b, h):
                st = state[b]
                if h == 0:
                    st["ot"] = otpool.tile([128, B], FP16, name="ot", tag="ot")
                o = accpool.tile([128, 512], F32, tag="o")
                st["o%d" % h] = o
                for (c, kind, js, je), (start, stop) in MM2_HALF[h]:
                    if kind == "mir":
                        rhs = st["am"][:, am_idx(c, js // 128), :]
                    else:
                        lo = a6_off(c, js)
                        rhs = st["a6"][:, lo:lo + (je - js)]
                    nc.tensor.matmul(
                        o[:, js - 512 * h:je - 512 * h],
                        lhsT=st["y"][:, c, :],
                        rhs=rhs,
                        start=start, stop=stop,
                    )

            def cast_half(b, h):
                st = state[b]
                nc.vector.tensor_scalar_mul(
                    st["ot"][:, h * 512:(h + 1) * 512],
                    st.pop("o%d" % h)[:], 1.0 / OSC
                )

            def tout(b):
                st = state[b]
                trd = trpool.tile([128, C, 128], FP16)
                nc.sync.dma_start_transpose(out=trd[:], in_=st.pop("ot")[:])
                st["trd"] = trd

            def tail(b):
                # out = (trd * OSC/B) * e_j  (one fused STT)
                st = state[b]
                of = ofpool.tile([128, C, D], F32)
                nc.vector.scalar_tensor_tensor(
                    out=of[:], in0=st["trd"][:], scalar=float(OSC / B),
                    in1=st["escB"][:], op0=MULT, op1=MULT,
                )
                st["of"] = of

            def store(b):
                st = state.pop(b)
                nc.sync.dma_start(out=fout_v[b], in_=st["of"][:])

            # software pipeline: iteration k handles load(k), prep/xtrans(k-1),
            # m1/act/mir(k-2), mm2/epilogue(k-4)
            for k in range(nb + 4):
                bl, bp, bm, be = k, k - 1, k - 2, k - 4
                if bl < nb:
                    load(bl)
                if 0 <= bp < nb:
                    prep(bp)
                if 0 <= bm < nb:
                    m1_tile(bm, 0)
                    act_tile(bm, 0)
                if 0 <= be < nb:
                    mm2_half(be, 0)
                if 0 <= bm < nb:
                    m1_tile(bm, 1)
                    act_tile(bm, 1)
                if 0 <= be < nb:
                    cast_half(be, 0)
                if 0 <= bm < nb:
                    m1_tile(bm, 2)
                    act_tile(bm, 2)
                if 0 <= be < nb:
                    mm2_half(be, 1)
                    cast_half(be, 1)
                    tout(be)
                if 0 <= bm < nb:
                    mir(bm)
                if 0 <= bp < nb:
                    xtrans(bp)
                    escalc(bp)
                    ymul(bp)
                if 0 <= be < nb:
                    tail(be)
                    store(be)

    nc.compile()
    return nc


_CACHE: dict[int, bacc.Bacc] = {}


def _get_nc(nb: int = NB_FULL) -> bacc.Bacc:
    if nb not in _CACHE:
        _CACHE[nb] = build(nb)
    return _CACHE[nb]


def run(features: np.ndarray, nc: bacc.Bacc | None = None, **spmd_kwargs):
    """Shard rows across 8 cores, run, gather. Returns (out, BassKernelResults)."""
    features = np.ascontiguousarray(features, dtype=np.float32)
    assert features.shape == (N_TOTAL, D)
    if nc is None:
        nc = _get_nc()
    core_ids = list(range(NCORES))
    shards = np.split(features, NCORES, axis=0)
    in_maps = [{"features": s} for s in shards]
    res = run_bass_kernel_spmd(nc, in_maps, core_ids, **spmd_kwargs)
    out = np.concatenate([res.results[i]["out"] for i in range(NCORES)], axis=0)
    return out, res


def kernel(features: np.ndarray) -> np.ndarray:
    out, _ = run(features)
    return out
